# revision 1
# baseline (speedup 1.0000x reference)
"""Distributed GAT (2-layer, heads=1) on 8 TRN2 NeuronCores.

Sharding: nodes partitioned across 8 cores by dst (12500/core, padded to
12544 = 98*128). Each core computes h = x_c @ W for its nodes, AllGathers
h|alpha_src rows, then processes its dst-partition edges (host-sorted by
dst, tiled so each 128-edge chunk covers one 128-dst window):
  per chunk: indirect-DMA gather of h|as rows by src, one-hot(dst) build,
  score = lrelu(as[src]+ad[dst]), ex = exp(score)*mask, and a single
  [128e x 128d].T @ [128e x (F+1)] matmul accumulates per-dst sums of
  (ex*h | ex) in PSUM; epilogue divides by the ex-sum (softmax folded into
  the aggregation), applies bias/relu, and fuses the layer-2 GEMM.
"""
import sys
sys.path.insert(0, '/opt/trn_rl_repo')
import numpy as np

import concourse.bass as bass
import concourse.bacc as bacc
import concourse.tile as tile
from concourse import mybir
from concourse.masks import make_identity
from concourse.bass_utils import run_bass_kernel_spmd

N_CORES = 8
N = 100000
NPC = N // N_CORES          # 12500 nodes per core
NT = 98                     # dst tiles per core
NPAD = NT * 128             # 12544 padded nodes per core
NFULL = N_CORES * NPAD      # 100352 padded global nodes
F1, H, O = 256, 64, 20
NEG_SLOPE = 0.2
AF = mybir.ActivationFunctionType
ALU = mybir.AluOpType
AX = mybir.AxisListType


def _prep_edges(edge_index):
    """Per-core edge arrays [128, NT*C]: padded-global src, dst window pos,
    valid mask. Edges sorted by dst; each dst-tile's edges padded to whole
    128-edge chunks; C = max chunks per tile over all cores/tiles."""
    # The appended self-loops (incl. the pad nodes' fake ones) are handled
    # separately: chunk 0 of every dst-tile is the 128 local rows
    # t*128..t*128+127, loaded by a sequential DMA on-device instead of an
    # indirect gather. edge_index itself is kept verbatim (it may contain
    # accidental (i,i) edges, which the reference counts separately).
    src = np.asarray(edge_index[0], dtype=np.int64)
    dst = np.asarray(edge_index[1], dtype=np.int64)

    per_core = []
    C = 1
    for c in range(N_CORES):
        m = (dst // NPC) == c
        s_c, d_c = src[m], dst[m] % NPC
        order = np.argsort(d_c, kind="stable")
        s_c, d_c = s_c[order], d_c[order]
        sp = (s_c // NPC) * NPAD + (s_c % NPC)       # padded-global src id
        t_c = d_c // 128
        counts = np.bincount(t_c, minlength=NT)
        per_core.append((sp, d_c, t_c, counts))

    allc = np.stack([pc[3] for pc in per_core])          # [cores, NT]
    Cts = 1 + np.ceil(allc.max(axis=0) / 128).astype(int)  # per-tile chunks
    off = np.zeros(NT, dtype=int)
    off[1:] = np.cumsum(Cts)[:-1]
    nch = int(Cts.sum())
    out = []
    for c, (sp, d_c, t_c, counts) in enumerate(per_core):
        srcs = np.zeros((128, nch), dtype=np.int32)
        dwin = np.zeros((128, nch), dtype=np.float32)
        mask = np.zeros((128, nch), dtype=np.float32)
        # chunk 0 per tile: the self-loop rows
        t_idx = np.arange(NT)
        srcs[:, off] = (c * NPAD + t_idx[None, :] * 128
                        + np.arange(128)[:, None]).astype(np.int32)
        dwin[:, off] = np.arange(128, dtype=np.float32)[:, None]
        mask[:, off] = 1.0
        # real (non-self) edges fill chunks 1..C_t-1 of each tile
        start = np.zeros(NT, dtype=np.int64)
        start[1:] = np.cumsum(counts)[:-1]
        i_t = np.arange(len(d_c)) - start[t_c]       # index within tile
        col = off[t_c] + 1 + i_t // 128
        row = i_t % 128
        srcs[row, col] = sp.astype(np.int32)
        dwin[row, col] = (d_c - t_c * 128).astype(np.float32)
        mask[row, col] = 1.0
        out.append((srcs, dwin, mask))
    return out, Cts.tolist(), off.tolist()


def _build(Cts, off):
    nch = int(sum(Cts))
    nc = bacc.Bacc("TRN2", target_bir_lowering=False, debug=False,
                   num_devices=N_CORES)
    dt = mybir.dt.float32
    xT = nc.dram_tensor("xT", [F1, NPAD], dt, kind="ExternalInput")
    w1 = nc.dram_tensor("w1", [F1, H], dt, kind="ExternalInput")
    w2 = nc.dram_tensor("w2", [H, O], dt, kind="ExternalInput")
    a1s = nc.dram_tensor("a1s", [H], dt, kind="ExternalInput")
    a1d = nc.dram_tensor("a1d", [H], dt, kind="ExternalInput")
    b1 = nc.dram_tensor("b1", [H], dt, kind="ExternalInput")
    a2s = nc.dram_tensor("a2s", [O], dt, kind="ExternalInput")
    a2d = nc.dram_tensor("a2d", [O], dt, kind="ExternalInput")
    b2 = nc.dram_tensor("b2", [O], dt, kind="ExternalInput")
    srcs = nc.dram_tensor("srcs", [128, nch], mybir.dt.int32, kind="ExternalInput")
    dwin = nc.dram_tensor("dwin", [128, nch], dt, kind="ExternalInput")
    maskd = nc.dram_tensor("mask", [128, nch], dt, kind="ExternalInput")
    iota = nc.dram_tensor("iota", [128, 128], dt, kind="ExternalInput")
    outp = nc.dram_tensor("outp", [NPAD, O], dt, kind="ExternalOutput")

    with tile.TileContext(nc) as tc:
        with tc.tile_pool(name="const", bufs=1) as cp, \
             tc.tile_pool(name="dram", bufs=1, space="DRAM") as dp, \
             tc.tile_pool(name="work", bufs=6) as wp, \
             tc.tile_pool(name="ps", bufs=2, space="PSUM") as pp:

            # ---- constants ----
            w1a = cp.tile([128, H], dt); nc.sync.dma_start(out=w1a[:], in_=w1[0:128, :])
            w1b = cp.tile([128, H], dt); nc.sync.dma_start(out=w1b[:], in_=w1[128:256, :])
            w2t = cp.tile([H, O], dt); nc.sync.dma_start(out=w2t[:], in_=w2[:])
            a1s_r = cp.tile([128, H], dt)
            nc.sync.dma_start(out=a1s_r[:], in_=a1s[None, :].to_broadcast([128, H]))
            a1d_r = cp.tile([128, H], dt)
            nc.sync.dma_start(out=a1d_r[:], in_=a1d[None, :].to_broadcast([128, H]))
            b1_r = cp.tile([128, H], dt)
            nc.sync.dma_start(out=b1_r[:], in_=b1[None, :].to_broadcast([128, H]))
            a2s_r = cp.tile([128, O], dt)
            nc.sync.dma_start(out=a2s_r[:], in_=a2s[None, :].to_broadcast([128, O]))
            a2d_r = cp.tile([128, O], dt)
            nc.sync.dma_start(out=a2d_r[:], in_=a2d[None, :].to_broadcast([128, O]))
            b2_r = cp.tile([128, O], dt)
            nc.sync.dma_start(out=b2_r[:], in_=b2[None, :].to_broadcast([128, O]))
            iot = cp.tile([128, 128], dt); nc.sync.dma_start(out=iot[:], in_=iota[:])
            ident = cp.tile([128, 128], dt); make_identity(nc, ident[:])
            srct = cp.tile([128, nch], mybir.dt.int32)
            nc.sync.dma_start(out=srct[:], in_=srcs[:])
            dwt = cp.tile([128, nch], dt); nc.sync.dma_start(out=dwt[:], in_=dwin[:])
            mkt = cp.tile([128, nch], dt); nc.sync.dma_start(out=mkt[:], in_=maskd[:])

            # ---- DRAM intermediates ----
            h1comb = dp.tile([NPAD, H + 1], dt)      # h | alpha_src
            ad1d = dp.tile([NPAD, 1], dt)
            h1full = dp.tile([NFULL, H + 1], dt)
            h2comb = dp.tile([NPAD, O + 1], dt)
            ad2d = dp.tile([NPAD, 1], dt)
            h2full = dp.tile([NFULL, O + 1], dt)

            # ---- phase 1: h1 = x @ W1, alpha_s/d ----
            for t in range(NT):
                k0 = wp.tile([128, 128], dt, tag="x0")
                nc.sync.dma_start(out=k0[:], in_=xT[0:128, t * 128:(t + 1) * 128])
                k1 = wp.tile([128, 128], dt, tag="x1")
                nc.sync.dma_start(out=k1[:], in_=xT[128:256, t * 128:(t + 1) * 128])
                hp = pp.tile([128, H], dt, tag="h1")
                nc.tensor.matmul(out=hp[:], lhsT=k0[:], rhs=w1a[:], start=True, stop=False)
                nc.tensor.matmul(out=hp[:], lhsT=k1[:], rhs=w1b[:], start=False, stop=True)
                hs = wp.tile([128, H], dt, tag="hs")
                nc.vector.tensor_copy(out=hs[:], in_=hp[:])
                tmp = wp.tile([128, H], dt, tag="tmp")
                nc.vector.tensor_mul(out=tmp[:], in0=hs[:], in1=a1s_r[:])
                asid = wp.tile([128, 2], dt, tag="as")
                nc.vector.reduce_sum(out=asid[:, 0:1], in_=tmp[:], axis=AX.X)
                nc.vector.tensor_mul(out=tmp[:], in0=hs[:], in1=a1d_r[:])
                nc.vector.reduce_sum(out=asid[:, 1:2], in_=tmp[:], axis=AX.X)
                nc.sync.dma_start(out=h1comb[t * 128:(t + 1) * 128, 0:H], in_=hs[:])
                nc.sync.dma_start(out=h1comb[t * 128:(t + 1) * 128, H:H + 1],
                                  in_=asid[:, 0:1])
                nc.sync.dma_start(out=ad1d[t * 128:(t + 1) * 128, :], in_=asid[:, 1:2])

            # ---- phase 2: all-gather h1|as ----
            nc.gpsimd.collective_compute(
                "AllGather", ALU.bypass, replica_groups=[list(range(N_CORES))],
                ins=[h1comb.opt()], outs=[h1full.opt()])

            def edge_layer(full_tbl, comb_local, ad_tbl, FW, comb_out, ad_out,
                           w_next, ans_r, and_r, bias_r, last):
                """One GAT edge phase (+ fused next-layer GEMM if not last)."""
                for t in range(NT):
                    adw = wp.tile([128, 128], dt, tag="adw")
                    nc.sync.dma_start(
                        out=adw[:],
                        in_=ad_tbl[t * 128:(t + 1) * 128, 0:1]
                        .rearrange("n one -> one n").to_broadcast([128, 128]))
                    ps = pp.tile([128, FW + 1], dt, tag="pe")
                    for k in range(Cts[t]):
                        cg = off[t] + k
                        gt = wp.tile([128, FW + 1], dt, tag="gt")
                        if k == 0:
                            # self-loop chunk: local contiguous rows
                            nc.sync.dma_start(
                                out=gt[:],
                                in_=comb_local[t * 128:(t + 1) * 128, :])
                        else:
                            nc.gpsimd.indirect_dma_start(
                                out=gt[:], out_offset=None, in_=full_tbl[:],
                                in_offset=bass.IndirectOffsetOnAxis(
                                    ap=srct[:, cg:cg + 1], axis=0))
                        oh = wp.tile([128, 128], dt, tag="oh")
                        nc.vector.tensor_tensor(
                            out=oh[:], in0=dwt[:, cg:cg + 1].to_broadcast([128, 128]),
                            in1=iot[:], op=ALU.is_equal)
                        sel = wp.tile([128, 128], dt, tag="sel")
                        nc.vector.tensor_mul(out=sel[:], in0=oh[:], in1=adw[:])
                        sc = wp.tile([128, 4], dt, tag="sc")
                        nc.vector.reduce_sum(out=sc[:, 0:1], in_=sel[:], axis=AX.X)
                        nc.vector.tensor_tensor(out=sc[:, 1:2], in0=gt[:, FW:FW + 1],
                                                in1=sc[:, 0:1], op=ALU.add)
                        nc.vector.tensor_scalar_mul(sc[:, 2:3], sc[:, 1:2], NEG_SLOPE)
                        nc.vector.tensor_tensor(out=sc[:, 1:2], in0=sc[:, 1:2],
                                                in1=sc[:, 2:3], op=ALU.max)
                        nc.scalar.activation(out=sc[:, 2:3], in_=sc[:, 1:2], func=AF.Exp)
                        nc.vector.tensor_mul(out=sc[:, 3:4], in0=sc[:, 2:3],
                                             in1=mkt[:, cg:cg + 1])
                        wt = wp.tile([128, FW + 1], dt, tag="wt")
                        nc.vector.tensor_scalar_mul(wt[:, 0:FW], gt[:, 0:FW], sc[:, 3:4])
                        nc.vector.tensor_copy(out=wt[:, FW:FW + 1], in_=sc[:, 3:4])
                        nc.tensor.matmul(out=ps[:], lhsT=oh[:], rhs=wt[:],
                                         start=(k == 0), stop=(k == Cts[t] - 1))
                    rec = wp.tile([128, 1], dt, tag="rec")
                    nc.vector.reciprocal(out=rec[:], in_=ps[:, FW:FW + 1])
                    o1 = wp.tile([128, FW], dt, tag="o1")
                    nc.vector.tensor_scalar_mul(o1[:], ps[:, 0:FW], rec[:])
                    nc.vector.tensor_add(out=o1[:], in0=o1[:], in1=bias_r[:])
                    if last:
                        nc.sync.dma_start(out=outp[t * 128:(t + 1) * 128, :], in_=o1[:])
                        continue
                    nc.scalar.activation(out=o1[:], in_=o1[:], func=AF.Relu)
                    trp = pp.tile([FW, 128], dt, tag="tr")
                    nc.tensor.transpose(out=trp[:], in_=o1[:], identity=ident[:])
                    o1T = wp.tile([FW, 128], dt, tag="o1T")
                    nc.vector.tensor_copy(out=o1T[:], in_=trp[:])
                    h2p = pp.tile([128, O], dt, tag="h2")
                    nc.tensor.matmul(out=h2p[:], lhsT=o1T[:], rhs=w_next[:],
                                     start=True, stop=True)
                    h2 = wp.tile([128, O], dt, tag="h2s")
                    nc.vector.tensor_copy(out=h2[:], in_=h2p[:])
                    tm2 = wp.tile([128, O], dt, tag="tm2")
                    nc.vector.tensor_mul(out=tm2[:], in0=h2[:], in1=ans_r[:])
                    as2 = wp.tile([128, 2], dt, tag="as2")
                    nc.vector.reduce_sum(out=as2[:, 0:1], in_=tm2[:], axis=AX.X)
                    nc.vector.tensor_mul(out=tm2[:], in0=h2[:], in1=and_r[:])
                    nc.vector.reduce_sum(out=as2[:, 1:2], in_=tm2[:], axis=AX.X)
                    nc.sync.dma_start(out=comb_out[t * 128:(t + 1) * 128, 0:O], in_=h2[:])
                    nc.sync.dma_start(out=comb_out[t * 128:(t + 1) * 128, O:O + 1],
                                      in_=as2[:, 0:1])
                    nc.sync.dma_start(out=ad_out[t * 128:(t + 1) * 128, :], in_=as2[:, 1:2])

            # ---- phase 3: edge layer 1 (+ fused layer-2 GEMM) ----
            edge_layer(h1full, h1comb, ad1d, H, h2comb, ad2d, w2t, a2s_r,
                       a2d_r, b1_r, last=False)

            # ---- phase 4: all-gather h2|as2 ----
            nc.gpsimd.collective_compute(
                "AllGather", ALU.bypass, replica_groups=[list(range(N_CORES))],
                ins=[h2comb.opt()], outs=[h2full.opt()])

            # ---- phase 5: edge layer 2 ----
            edge_layer(h2full, h2comb, ad2d, O, None, None, None, None, None,
                       b2_r, last=True)

    nc.compile()
    return nc


def kernel(x, edge_index, W1, a1_src, a1_dst, b1, W2, a2_src, a2_dst, b2):
    x = np.asarray(x, dtype=np.float32)
    edge_arrays, Cts, off = _prep_edges(np.asarray(edge_index))
    nc = _build(Cts, off)

    iota_np = np.tile(np.arange(128, dtype=np.float32), (128, 1))
    common = dict(
        w1=np.asarray(W1, np.float32), w2=np.asarray(W2, np.float32),
        a1s=np.asarray(a1_src, np.float32), a1d=np.asarray(a1_dst, np.float32),
        b1=np.asarray(b1, np.float32), a2s=np.asarray(a2_src, np.float32),
        a2d=np.asarray(a2_dst, np.float32), b2=np.asarray(b2, np.float32),
        iota=iota_np,
    )
    in_maps = []
    for c in range(N_CORES):
        srcs, dwin, mask = edge_arrays[c]
        xT = np.zeros((F1, NPAD), np.float32)
        xT[:, :NPC] = x[c * NPC:(c + 1) * NPC].T
        in_maps.append(dict(common, xT=xT, srcs=srcs, dwin=dwin, mask=mask))

    global _LAST_NC, _LAST_INMAPS
    _LAST_NC, _LAST_INMAPS = nc, in_maps
    res = run_bass_kernel_spmd(nc, in_maps, core_ids=list(range(N_CORES)))
    out = np.concatenate(
        [res.results[c]["outp"][:NPC] for c in range(N_CORES)], axis=0)
    return out.astype(np.float32)



# revision 13
# speedup vs baseline: 1.5081x; 1.5081x over previous
"""Distributed GAT (2-layer, heads=1) on 8 TRN2 NeuronCores — v2.

Nodes partitioned by dst across 8 cores (12500/core, padded 12544=98*128).
Per layer: each core computes a combined bf16 row table
[h(bf16) | a_src-logit(fp32 in 2 bf16 slots) | 1.0 | pad] = 256B/row,
AllGathers it, then processes its dst tiles. Edge gathering uses batched
gpsimd dma_gather (one SWDGE instruction per (8-tile group x int16 row
range) instead of one indirect DMA per 128-edge chunk). One-hot scatter
matrices are host-staged bf16 and drive both the per-dst PSUM-accumulated
scatter matmul (with softmax denominator as a folded 'ones' column) and a
fused scalar_tensor_tensor ad-select+row-reduce. Scores/exp are computed
group-wide; per-chunk weighting runs on the scalar engine via
activation(scale=ex). Self-loop chunks load sequentially from the local
table and scatter through a staged identity.
"""
import sys
sys.path.insert(0, '/opt/trn_rl_repo')
import numpy as np
import ml_dtypes

import concourse.bass as bass
import concourse.bacc as bacc
import concourse.tile as tile
from concourse import mybir
from concourse.bass_utils import run_bass_kernel_spmd
from concourse.library_config import mlp

N_CORES = 8
N = 100000
NPC = N // N_CORES          # 12500 nodes per core
NT = 98                     # dst tiles per core
NPAD = NT * 128             # 12544 padded nodes per core
NFULL = N_CORES * NPAD      # 100352 padded global nodes
RANGE = NFULL // 4          # 25088 rows per int16-addressable table range
F1, H, O = 256, 64, 20
EW = 128                    # bf16 slots per table row (256B)
NEG_SLOPE = 0.2
GT = 6                      # tiles per group
AF = mybir.ActivationFunctionType
ALU = mybir.AluOpType
BF = mybir.dt.bfloat16
F32 = mybir.dt.float32


def _prep_edges(edge_index):
    """Group real edges per core by (dst tile, src range); chunk counts are
    maxed over cores so one SPMD program fits all. Returns per-core staged
    arrays (idx16 wrapped layout, bf16 one-hot) + the global chunk layout."""
    src = np.asarray(edge_index[0], dtype=np.int64)
    dst = np.asarray(edge_index[1], dtype=np.int64)
    sp_all = (src // NPC) * NPAD + (src % NPC)      # padded-global src id

    per_core = []
    counts = np.zeros((N_CORES, NT, 4), dtype=np.int64)
    for c in range(N_CORES):
        m = (dst // NPC) == c
        sp, ld = sp_all[m], (dst[m] % NPC).astype(np.int64)
        t = ld // 128
        w = ld % 128
        r = sp // RANGE
        order = np.lexsort((sp, r, t))
        sp, t, w, r = sp[order], t[order], w[order], r[order]
        np.add.at(counts[c], (t, r), 1)
        per_core.append((sp, t, w, r))

    K = np.ceil(counts.max(axis=0) / 128).astype(np.int64)   # [NT, 4] chunks
    # global chunk layout: groups of GT tiles; within a group chunks are
    # ordered (range, tile, k) so each range is one contiguous gather span.
    groups = []
    col = 0
    for g0 in range(0, NT, GT):
        tiles = list(range(g0, min(g0 + GT, NT)))
        spans = []          # per range: (start_col, n_chunks)
        tile_cols = {t: [] for t in tiles}
        start_col = col
        for r in range(4):
            s = col
            for t in tiles:
                for k in range(K[t][r]):
                    tile_cols[t].append(col)
                    col += 1
            spans.append((s, col - s))
        groups.append(dict(tiles=tiles, spans=spans, tile_cols=tile_cols,
                           start=start_col, n=col - start_col))
    totch = col

    staged = []
    for c in range(N_CORES):
        sp, t, w, r = per_core[c]
        # chunk-slot assignment: within (t, r), edges fill chunks in order
        k_off = np.zeros((NT, 4), dtype=np.int64)
        k_off[:, 1:] = np.cumsum(K, axis=1)[:, :-1]
        base = np.zeros((NT, 4), dtype=np.int64)    # first col of (t,r)
        pos_in = np.zeros(len(sp), dtype=np.int64)  # index within (t,r)
        # compute start positions per (t, r) via sorted order
        key = t * 4 + r
        starts = np.zeros(NT * 4, dtype=np.int64)
        cnt = np.bincount(key, minlength=NT * 4)
        starts[1:] = np.cumsum(cnt)[:-1]
        pos_in = np.arange(len(sp)) - starts[key]
        # column of each edge: group layout lookup
        colmap = np.zeros((NT, 4), dtype=np.int64)
        for g in groups:
            for ti in g["tiles"]:
                cols = g["tile_cols"][ti]
                # cols are ordered r-major with K[ti][r] entries each
                o = 0
                for r_ in range(4):
                    colmap[ti, r_] = cols[o] if K[ti][r_] > 0 else 0
                    o += K[ti][r_]
        ecol = colmap[t, r] + pos_in // 128
        erow = pos_in % 128

        idx16 = np.zeros((128, totch * 8), dtype=np.int16)
        rel = (sp - r * RANGE).astype(np.int16)
        for s in range(8):
            idx16[16 * s + erow % 16, ecol * 8 + erow // 16] = rel
        oh = np.zeros((128, totch * 128), dtype=ml_dtypes.bfloat16)
        oh[erow, ecol * 128 + w] = 1.0
        staged.append((idx16, oh))
    return staged, groups, totch


def _build(groups, totch):
    nc = bacc.Bacc("TRN2", target_bir_lowering=False, debug=False,
                   num_devices=N_CORES)
    NGRP = len(groups)
    xT = nc.dram_tensor("xT", [F1, NPAD], BF, kind="ExternalInput")
    w1 = nc.dram_tensor("w1", [F1, H], BF, kind="ExternalInput")
    w2 = nc.dram_tensor("w2", [H, O], BF, kind="ExternalInput")
    a1s = nc.dram_tensor("a1s", [H], F32, kind="ExternalInput")
    a1d = nc.dram_tensor("a1d", [H], F32, kind="ExternalInput")
    b1 = nc.dram_tensor("b1", [H], F32, kind="ExternalInput")
    a2s = nc.dram_tensor("a2s", [O], F32, kind="ExternalInput")
    a2d = nc.dram_tensor("a2d", [O], F32, kind="ExternalInput")
    b2 = nc.dram_tensor("b2", [O], F32, kind="ExternalInput")
    idxs = nc.dram_tensor("idxs", [128, totch * 8], mybir.dt.int16,
                          kind="ExternalInput")
    ohd = nc.dram_tensor("ohd", [128, totch * 128], BF, kind="ExternalInput")
    identd = nc.dram_tensor("identd", [128, 128], BF, kind="ExternalInput")
    outp = nc.dram_tensor("outp", [NPAD, O], F32, kind="ExternalOutput")

    with tile.TileContext(nc) as tc:
        with tc.tile_pool(name="const", bufs=1) as cp, \
             tc.tile_pool(name="dram", bufs=1, space="DRAM") as dp, \
             tc.tile_pool(name="xp", bufs=2) as xp, \
             tc.tile_pool(name="ohp", bufs=2) as ohp, \
             tc.tile_pool(name="gp", bufs=2) as gp, \
             tc.tile_pool(name="ep", bufs=2) as ep, \
             tc.tile_pool(name="wp", bufs=4) as wp, \
             tc.tile_pool(name="ps", bufs=2, space="PSUM") as pp:

            nc.gpsimd.load_library(mlp)
            tc.no_sync_barrier()

            # ---- constants ----
            w1a = cp.tile([128, H], BF); nc.sync.dma_start(out=w1a[:], in_=w1[0:128, :])
            w1b = cp.tile([128, H], BF); nc.sync.dma_start(out=w1b[:], in_=w1[128:256, :])
            w2t = cp.tile([H, O], BF); nc.sync.dma_start(out=w2t[:], in_=w2[:])
            def brow(name, vec, n):
                tl = cp.tile([128, n], F32, tag=name)
                nc.sync.dma_start(out=tl[:], in_=vec[None, :].to_broadcast([128, n]))
                return tl
            a1s_r = brow("a1s", a1s, H); a1d_r = brow("a1d", a1d, H)
            b1_r = brow("b1", b1, H)
            a2s_r = brow("a2s", a2s, O); a2d_r = brow("a2d", a2d, O)
            b2_r = brow("b2", b2, O)
            identb = cp.tile([128, 128], BF)
            nc.sync.dma_start(out=identb[:], in_=identd[:])
            idxt = cp.tile([128, totch * 8], mybir.dt.int16)
            nc.sync.dma_start(out=idxt[:], in_=idxs[:])

            # ---- DRAM intermediates ----
            comb1_l = dp.tile([NPAD, EW], BF)
            comb1_f = dp.tile([NFULL, EW], BF)
            ad1_l = dp.tile([NPAD, 1], F32)
            comb2_l = dp.tile([NPAD, EW], BF)
            comb2_f = dp.tile([NFULL, EW], BF)
            ad2_l = dp.tile([NPAD, 1], F32)

            # ---- phase 1: h1 = x @ W1 (+ logits), packed bf16 rows ----
            for g in groups:
                tiles = g["tiles"]; gt = len(tiles); g0 = tiles[0]
                xs0 = xp.tile([128, gt * 128], BF, tag="xs0")
                nc.sync.dma_start(out=xs0[:], in_=xT[0:128, g0 * 128:(g0 + gt) * 128])
                xs1 = xp.tile([128, gt * 128], BF, tag="xs1")
                nc.sync.dma_start(out=xs1[:], in_=xT[128:256, g0 * 128:(g0 + gt) * 128])
                gcomb = xp.tile([128, gt * EW], BF, tag="gcomb")
                gcf32 = gcomb[:].bitcast(F32)
                adg = xp.tile([128, gt], F32, tag="adg")
                nc.vector.memset(gcomb[:], 0.0)
                nc.vector.memset(
                    gcomb[:].rearrange("p (t k) -> p t k", k=EW)[:, :, H + 2:H + 3], 1.0)
                for i in range(gt):
                    hp = pp.tile([128, H], F32, tag="hp")
                    nc.tensor.matmul(out=hp[:], lhsT=xs0[:, i * 128:(i + 1) * 128],
                                     rhs=w1a[:], start=True, stop=False)
                    nc.tensor.matmul(out=hp[:], lhsT=xs1[:, i * 128:(i + 1) * 128],
                                     rhs=w1b[:], start=False, stop=True)
                    nc.scalar.activation(out=gcomb[:, i * EW:i * EW + H], in_=hp[:],
                                         func=AF.Copy)
                    scr = wp.tile([128, H], F32, tag="scr")
                    ascol = wp.tile([128, 1], F32, tag="ascol")
                    nc.vector.scalar_tensor_tensor(
                        out=scr[:], in0=hp[:], scalar=1.0, in1=a1s_r[:],
                        op0=ALU.mult, op1=ALU.mult, accum_out=ascol[:])
                    nc.vector.scalar_tensor_tensor(
                        out=scr[:], in0=hp[:], scalar=1.0, in1=a1d_r[:],
                        op0=ALU.mult, op1=ALU.mult, accum_out=adg[:, i:i + 1])
                    nc.vector.tensor_copy(out=gcf32[:, i * (EW // 2) + H // 2:
                                                    i * (EW // 2) + H // 2 + 1],
                                          in_=ascol[:])
                nc.sync.dma_start(
                    out=comb1_l[:].rearrange("(t p) k -> p t k", p=128)
                    [:, g0:g0 + gt, :], in_=gcomb[:])
                nc.sync.dma_start(
                    out=ad1_l[:].rearrange("(t p) one -> p t one", p=128)
                    [:, g0:g0 + gt, :], in_=adg[:])

            # ---- phase 2: all-gather layer-1 table ----
            nc.gpsimd.collective_compute(
                "AllGather", ALU.bypass, replica_groups=[list(range(N_CORES))],
                ins=[comb1_l[:].opt()], outs=[comb1_f[:].opt()])

            def edge_layer(comb_f, comb_l, ad_l, FW, last,
                           ans_r, and_r, bias_r, comb_out, ad_out):
                FWU = FW + 3        # h | as(2) | one
                for g in groups:
                    tiles = g["tiles"]; gt = len(tiles); g0 = tiles[0]
                    ngc = g["n"]; c0 = g["start"]
                    ohg = ohp.tile([128, ngc * 128], BF, tag="ohg")
                    nc.sync.dma_start(out=ohg[:],
                                      in_=ohd[:, c0 * 128:(c0 + ngc) * 128])
                    gbuf = gp.tile([128, ngc * 128], BF, tag="gbuf")
                    for (s_r, n_r), rbase in zip(g["spans"],
                                                 range(0, NFULL, RANGE)):
                        # HW limit: ~1024 idx per dma_gather (128B/partition
                        # of wrapped idx data); split spans into <=8-chunk ops
                        for p0 in range(0, n_r, 8):
                            pn = min(8, n_r - p0)
                            s_p = s_r + p0
                            gview = gbuf[:, (s_p - c0) * 128:(s_p - c0 + pn) * 128] \
                                .rearrange("p (c k) -> p c k", k=128)
                            nc.gpsimd.dma_gather(
                                gview, comb_f[rbase:rbase + RANGE, :],
                                idxt[:, s_p * 8:(s_p + pn) * 8],
                                pn * 128, pn * 128, EW)
                    # self-loop rows: sequential load of local table rows
                    gself = gp.tile([128, gt * EW], BF, tag="gself")
                    nc.sync.dma_start(
                        out=gself[:],
                        in_=comb_l[:].rearrange("(t p) k -> p t k", p=128)
                        [:, g0:g0 + gt, :])
                    # ad tiles: [128, gt*128] broadcast (free axis) + diag col
                    adw = ep.tile([128, gt * 128], F32, tag="adw")
                    nc.sync.dma_start(
                        out=adw[:],
                        in_=ad_l[g0 * 128:(g0 + gt) * 128, 0:1]
                        .rearrange("n one -> one n").to_broadcast([128, gt * 128]))
                    adc = ep.tile([128, gt], F32, tag="adc")
                    nc.sync.dma_start(
                        out=adc[:],
                        in_=ad_l[:].rearrange("(t p) one -> p t one", p=128)
                        [:, g0:g0 + gt, :])
                    # ad per edge (gathered chunks): fused onehot*ad + rowsum
                    ade = ep.tile([128, ngc], F32, tag="ade")
                    for t_i, t in enumerate(tiles):
                        for c in g["tile_cols"][t]:
                            osel = wp.tile([128, 128], F32, tag="osel")
                            nc.vector.scalar_tensor_tensor(
                                out=osel[:], in0=ohg[:, (c - c0) * 128:(c - c0 + 1) * 128],
                                scalar=1.0, in1=adw[:, t_i * 128:(t_i + 1) * 128],
                                op0=ALU.mult, op1=ALU.mult,
                                accum_out=ade[:, c - c0:c - c0 + 1])
                    # scores -> ex  (gathered chunks)
                    gf32 = gbuf[:].bitcast(F32).rearrange(
                        "p (c k) -> p c k", k=EW // 2)
                    asv = gf32[:, :, FW // 2:FW // 2 + 1].squeeze(2)
                    et = ep.tile([128, ngc], F32, tag="et")
                    nc.vector.tensor_tensor(out=et[:], in0=asv, in1=ade[:], op=ALU.add)
                    lrt = ep.tile([128, ngc], F32, tag="lrt")
                    nc.vector.scalar_tensor_tensor(
                        out=lrt[:], in0=et[:], scalar=NEG_SLOPE, in1=et[:],
                        op0=ALU.mult, op1=ALU.max)
                    ext = ep.tile([128, ngc], F32, tag="ext")
                    nc.scalar.activation(out=ext[:], in_=lrt[:], func=AF.Exp)
                    # scores -> ex  (self chunks)
                    gsf32 = gself[:].bitcast(F32).rearrange(
                        "p (t k) -> p t k", k=EW // 2)
                    asv_s = gsf32[:, :, FW // 2:FW // 2 + 1].squeeze(2)
                    ets = ep.tile([128, gt], F32, tag="ets")
                    nc.vector.tensor_tensor(out=ets[:], in0=asv_s, in1=adc[:], op=ALU.add)
                    lrs = ep.tile([128, gt], F32, tag="lrs")
                    nc.vector.scalar_tensor_tensor(
                        out=lrs[:], in0=ets[:], scalar=NEG_SLOPE, in1=ets[:],
                        op0=ALU.mult, op1=ALU.max)
                    exs = ep.tile([128, gt], F32, tag="exs")
                    nc.scalar.activation(out=exs[:], in_=lrs[:], func=AF.Exp)

                    # per-tile: weight + scatter-accumulate + epilogue
                    if last:
                        gout = ep.tile([128, gt * O], F32, tag="gout")
                    else:
                        gc2 = ep.tile([128, gt * EW], BF, tag="gc2")
                        gc2f = gc2[:].bitcast(F32)
                        adg2 = ep.tile([128, gt], F32, tag="adg2")
                        nc.vector.memset(gc2[:], 0.0)
                        nc.vector.memset(
                            gc2[:].rearrange("p (t k) -> p t k", k=EW)
                            [:, :, O + 2:O + 3], 1.0)
                    for t_i, t in enumerate(tiles):
                        ps = pp.tile([128, FWU], F32, tag="pe")
                        wts = wp.tile([128, FWU], BF, tag="wts")
                        nc.scalar.activation(
                            out=wts[:], in_=gself[:, t_i * EW:t_i * EW + FWU],
                            func=AF.Copy, scale=exs[:, t_i:t_i + 1])
                        nc.tensor.matmul(out=ps[:], lhsT=identb[:], rhs=wts[:],
                                         start=True, stop=False)
                        cols = g["tile_cols"][t]
                        for j, c in enumerate(cols):
                            wt = wp.tile([128, FWU], BF, tag="wt")
                            nc.scalar.activation(
                                out=wt[:], in_=gbuf[:, (c - c0) * 128:(c - c0) * 128 + FWU],
                                func=AF.Copy, scale=ext[:, c - c0:c - c0 + 1])
                            nc.tensor.matmul(
                                out=ps[:], lhsT=ohg[:, (c - c0) * 128:(c - c0 + 1) * 128],
                                rhs=wt[:], start=False, stop=(j == len(cols) - 1))
                        # epilogue
                        rec = wp.tile([128, 1], F32, tag="rec")
                        nc.vector.reciprocal(out=rec[:], in_=ps[:, FWU - 1:FWU])
                        if last:
                            nc.vector.scalar_tensor_tensor(
                                out=gout[:, t_i * O:(t_i + 1) * O], in0=ps[:, 0:FW],
                                scalar=rec[:], in1=bias_r[:],
                                op0=ALU.mult, op1=ALU.add)
                            continue
                        o1 = wp.tile([128, FW], F32, tag="o1")
                        nc.vector.scalar_tensor_tensor(
                            out=o1[:], in0=ps[:, 0:FW], scalar=rec[:],
                            in1=bias_r[:], op0=ALU.mult, op1=ALU.add)
                        o1b = wp.tile([128, FW], BF, tag="o1b")
                        nc.scalar.activation(out=o1b[:], in_=o1[:], func=AF.Relu)
                        trp = pp.tile([FW, 128], BF, tag="tr")
                        nc.tensor.transpose(out=trp[:], in_=o1b[:], identity=identb[:])
                        o1T = wp.tile([FW, 128], BF, tag="o1T")
                        nc.vector.tensor_copy(out=o1T[:], in_=trp[:])
                        h2p = pp.tile([128, O], F32, tag="h2")
                        nc.tensor.matmul(out=h2p[:], lhsT=o1T[:], rhs=w2t[:],
                                         start=True, stop=True)
                        nc.scalar.activation(out=gc2[:, t_i * EW:t_i * EW + O],
                                             in_=h2p[:], func=AF.Copy)
                        scr2 = wp.tile([128, O], F32, tag="scr2")
                        as2 = wp.tile([128, 1], F32, tag="as2")
                        nc.vector.scalar_tensor_tensor(
                            out=scr2[:], in0=h2p[:], scalar=1.0, in1=ans_r[:],
                            op0=ALU.mult, op1=ALU.mult, accum_out=as2[:])
                        nc.vector.scalar_tensor_tensor(
                            out=scr2[:], in0=h2p[:], scalar=1.0, in1=and_r[:],
                            op0=ALU.mult, op1=ALU.mult,
                            accum_out=adg2[:, t_i:t_i + 1])
                        nc.vector.tensor_copy(
                            out=gc2f[:, t_i * (EW // 2) + O // 2:
                                     t_i * (EW // 2) + O // 2 + 1], in_=as2[:])
                    if last:
                        nc.sync.dma_start(
                            out=outp[:].rearrange("(t p) k -> p t k", p=128)
                            [:, g0:g0 + gt, :], in_=gout[:])
                    else:
                        nc.sync.dma_start(
                            out=comb_out[:].rearrange("(t p) k -> p t k", p=128)
                            [:, g0:g0 + gt, :], in_=gc2[:])
                        nc.sync.dma_start(
                            out=ad_out[:].rearrange("(t p) one -> p t one", p=128)
                            [:, g0:g0 + gt, :], in_=adg2[:])

            # ---- phase 3: edge layer 1 (fused layer-2 GEMM) ----
            edge_layer(comb1_f, comb1_l, ad1_l, H, False,
                       a2s_r, a2d_r, b1_r, comb2_l, ad2_l)

            # ---- phase 4: all-gather layer-2 table ----
            nc.gpsimd.collective_compute(
                "AllGather", ALU.bypass, replica_groups=[list(range(N_CORES))],
                ins=[comb2_l[:].opt()], outs=[comb2_f[:].opt()])

            # ---- phase 5: edge layer 2 ----
            edge_layer(comb2_f, comb2_l, ad2_l, O, True,
                       None, None, b2_r, None, None)

    nc.compile()
    return nc


def kernel(x, edge_index, W1, a1_src, a1_dst, b1, W2, a2_src, a2_dst, b2):
    x = np.asarray(x, dtype=np.float32)
    staged, groups, totch = _prep_edges(np.asarray(edge_index))
    nc = _build(groups, totch)

    ident = np.eye(128, dtype=ml_dtypes.bfloat16)
    common = dict(
        w1=np.asarray(W1, np.float32).astype(ml_dtypes.bfloat16),
        w2=np.asarray(W2, np.float32).astype(ml_dtypes.bfloat16),
        a1s=np.asarray(a1_src, np.float32), a1d=np.asarray(a1_dst, np.float32),
        b1=np.asarray(b1, np.float32), a2s=np.asarray(a2_src, np.float32),
        a2d=np.asarray(a2_dst, np.float32), b2=np.asarray(b2, np.float32),
        identd=ident,
    )
    in_maps = []
    for c in range(N_CORES):
        idx16, oh = staged[c]
        xT = np.zeros((F1, NPAD), ml_dtypes.bfloat16)
        xT[:, :NPC] = x[c * NPC:(c + 1) * NPC].T.astype(ml_dtypes.bfloat16)
        in_maps.append(dict(common, xT=xT, idxs=idx16, ohd=oh))

    global _LAST_NC, _LAST_INMAPS
    _LAST_NC, _LAST_INMAPS = nc, in_maps
    res = run_bass_kernel_spmd(nc, in_maps, core_ids=list(range(N_CORES)))
    out = np.concatenate(
        [res.results[c]["outp"][:NPC] for c in range(N_CORES)], axis=0)
    return out.astype(np.float32)


# revision 16
# speedup vs baseline: 2.4825x; 1.6461x over previous
"""Distributed GAT (2-layer, heads=1) on 8 TRN2 NeuronCores — v2.

Nodes partitioned by dst across 8 cores (12500/core, padded 12544=98*128).
Per layer: each core computes a combined bf16 row table
[h(bf16) | a_src-logit(fp32 in 2 bf16 slots) | 1.0 | pad] = 256B/row,
AllGathers it, then processes its dst tiles. Edge gathering uses batched
gpsimd dma_gather (one SWDGE instruction per (8-tile group x int16 row
range) instead of one indirect DMA per 128-edge chunk). One-hot scatter
matrices are host-staged bf16 and drive both the per-dst PSUM-accumulated
scatter matmul (with softmax denominator as a folded 'ones' column) and a
fused scalar_tensor_tensor ad-select+row-reduce. Scores/exp are computed
group-wide; per-chunk weighting runs on the scalar engine via
activation(scale=ex). Self-loop chunks load sequentially from the local
table and scatter through a staged identity.
"""
import sys
sys.path.insert(0, '/opt/trn_rl_repo')
import numpy as np
import ml_dtypes

import concourse.bass as bass
import concourse.bacc as bacc
import concourse.tile as tile
from concourse import mybir
from concourse.bass_utils import run_bass_kernel_spmd
from concourse.library_config import mlp

N_CORES = 8
N = 100000
NPC = N // N_CORES          # 12500 nodes per core
NT = 98                     # dst tiles per core
NPAD = NT * 128             # 12544 padded nodes per core
NFULL = N_CORES * NPAD      # 100352 padded global nodes
RANGE = NFULL // 4          # 25088 rows per int16-addressable table range
F1, H, O = 256, 64, 20
EW = 128                    # bf16 slots per table row (256B)
NEG_SLOPE = 0.2
GT = 6                      # tiles per group
AF = mybir.ActivationFunctionType
ALU = mybir.AluOpType
BF = mybir.dt.bfloat16
F32 = mybir.dt.float32


def _prep_edges(edge_index):
    """Group real edges per core by (dst tile, src range); chunk counts are
    maxed over cores so one SPMD program fits all. Returns per-core staged
    arrays (idx16 wrapped layout, bf16 one-hot) + the global chunk layout."""
    src = np.asarray(edge_index[0], dtype=np.int64)
    dst = np.asarray(edge_index[1], dtype=np.int64)
    sp_all = (src // NPC) * NPAD + (src % NPC)      # padded-global src id

    per_core = []
    counts = np.zeros((N_CORES, NT, 4), dtype=np.int64)
    for c in range(N_CORES):
        m = (dst // NPC) == c
        sp, ld = sp_all[m], (dst[m] % NPC).astype(np.int64)
        t = ld // 128
        w = ld % 128
        r = sp // RANGE
        order = np.lexsort((sp, r, t))
        sp, t, w, r = sp[order], t[order], w[order], r[order]
        np.add.at(counts[c], (t, r), 1)
        per_core.append((sp, t, w, r))

    K = np.ceil(counts.max(axis=0) / 128).astype(np.int64)   # [NT, 4] chunks
    # global chunk layout: groups of GT tiles; within a group chunks are
    # ordered (range, tile, k) so each range is one contiguous gather span.
    groups = []
    col = 0
    for g0 in range(0, NT, GT):
        tiles = list(range(g0, min(g0 + GT, NT)))
        spans = []          # per range: (start_col, n_chunks)
        tile_cols = {t: [] for t in tiles}
        start_col = col
        for r in range(4):
            s = col
            for t in tiles:
                for k in range(K[t][r]):
                    tile_cols[t].append(col)
                    col += 1
            spans.append((s, col - s))
        groups.append(dict(tiles=tiles, spans=spans, tile_cols=tile_cols,
                           start=start_col, n=col - start_col))
    totch = col

    staged = []
    for c in range(N_CORES):
        sp, t, w, r = per_core[c]
        # chunk-slot assignment: within (t, r), edges fill chunks in order
        k_off = np.zeros((NT, 4), dtype=np.int64)
        k_off[:, 1:] = np.cumsum(K, axis=1)[:, :-1]
        base = np.zeros((NT, 4), dtype=np.int64)    # first col of (t,r)
        pos_in = np.zeros(len(sp), dtype=np.int64)  # index within (t,r)
        # compute start positions per (t, r) via sorted order
        key = t * 4 + r
        starts = np.zeros(NT * 4, dtype=np.int64)
        cnt = np.bincount(key, minlength=NT * 4)
        starts[1:] = np.cumsum(cnt)[:-1]
        pos_in = np.arange(len(sp)) - starts[key]
        # column of each edge: group layout lookup
        colmap = np.zeros((NT, 4), dtype=np.int64)
        for g in groups:
            for ti in g["tiles"]:
                cols = g["tile_cols"][ti]
                # cols are ordered r-major with K[ti][r] entries each
                o = 0
                for r_ in range(4):
                    colmap[ti, r_] = cols[o] if K[ti][r_] > 0 else 0
                    o += K[ti][r_]
        ecol = colmap[t, r] + pos_in // 128
        erow = pos_in % 128

        idx16 = np.zeros((128, totch * 8), dtype=np.int16)
        rel = (sp - r * RANGE).astype(np.int16)
        for s in range(8):
            idx16[16 * s + erow % 16, ecol * 8 + erow // 16] = rel
        oh = np.zeros((128, totch * 128), dtype=ml_dtypes.bfloat16)
        oh[erow, ecol * 128 + w] = 1.0
        staged.append((idx16, oh))
    return staged, groups, totch


def _build(groups, totch):
    nc = bacc.Bacc("TRN2", target_bir_lowering=False, debug=False,
                   num_devices=N_CORES, num_swdge_queues=4)
    NGRP = len(groups)
    xT = nc.dram_tensor("xT", [F1, NPAD], BF, kind="ExternalInput")
    w1 = nc.dram_tensor("w1", [F1, H], BF, kind="ExternalInput")
    w2 = nc.dram_tensor("w2", [H, O], BF, kind="ExternalInput")
    a1s = nc.dram_tensor("a1s", [H], F32, kind="ExternalInput")
    a1d = nc.dram_tensor("a1d", [H], F32, kind="ExternalInput")
    b1 = nc.dram_tensor("b1", [H], F32, kind="ExternalInput")
    a2s = nc.dram_tensor("a2s", [O], F32, kind="ExternalInput")
    a2d = nc.dram_tensor("a2d", [O], F32, kind="ExternalInput")
    b2 = nc.dram_tensor("b2", [O], F32, kind="ExternalInput")
    idxs = nc.dram_tensor("idxs", [128, totch * 8], mybir.dt.int16,
                          kind="ExternalInput")
    ohd = nc.dram_tensor("ohd", [128, totch * 128], BF, kind="ExternalInput")
    identd = nc.dram_tensor("identd", [128, 128], BF, kind="ExternalInput")
    outp = nc.dram_tensor("outp", [NPAD, O], F32, kind="ExternalOutput")

    with tile.TileContext(nc) as tc:
        with tc.tile_pool(name="const", bufs=1) as cp, \
             tc.tile_pool(name="dram", bufs=1, space="DRAM") as dp, \
             tc.tile_pool(name="xp", bufs=2) as xp, \
             tc.tile_pool(name="ohp", bufs=2) as ohp, \
             tc.tile_pool(name="gp", bufs=2) as gp, \
             tc.tile_pool(name="ep", bufs=2) as ep, \
             tc.tile_pool(name="wp", bufs=4) as wp, \
             tc.tile_pool(name="ps", bufs=2, space="PSUM") as pp:

            nc.gpsimd.load_library(mlp)
            tc.no_sync_barrier()

            # ---- constants ----
            w1a = cp.tile([128, H], BF); nc.sync.dma_start(out=w1a[:], in_=w1[0:128, :])
            w1b = cp.tile([128, H], BF); nc.sync.dma_start(out=w1b[:], in_=w1[128:256, :])
            w2t = cp.tile([H, O], BF); nc.sync.dma_start(out=w2t[:], in_=w2[:])
            def brow(name, vec, n):
                tl = cp.tile([128, n], F32, tag=name)
                nc.sync.dma_start(out=tl[:], in_=vec[None, :].to_broadcast([128, n]))
                return tl
            a1s_r = brow("a1s", a1s, H); a1d_r = brow("a1d", a1d, H)
            b1_r = brow("b1", b1, H)
            a2s_r = brow("a2s", a2s, O); a2d_r = brow("a2d", a2d, O)
            b2_r = brow("b2", b2, O)
            identb = cp.tile([128, 128], BF)
            nc.sync.dma_start(out=identb[:], in_=identd[:])
            idxt = cp.tile([128, totch * 8], mybir.dt.int16)
            nc.sync.dma_start(out=idxt[:], in_=idxs[:])

            # ---- DRAM intermediates ----
            comb1_l = dp.tile([NPAD, EW], BF)
            comb1_f = dp.tile([NFULL, EW], BF)
            ad1_l = dp.tile([NPAD, 1], F32)
            comb2_l = dp.tile([NPAD, EW], BF)
            comb2_f = dp.tile([NFULL, EW], BF)
            ad2_l = dp.tile([NPAD, 1], F32)

            # ---- phase 1: h1 = x @ W1 (+ logits), packed bf16 rows ----
            for g in groups:
                tiles = g["tiles"]; gt = len(tiles); g0 = tiles[0]
                xs0 = xp.tile([128, gt * 128], BF, tag="xs0")
                nc.sync.dma_start(out=xs0[:], in_=xT[0:128, g0 * 128:(g0 + gt) * 128])
                xs1 = xp.tile([128, gt * 128], BF, tag="xs1")
                nc.sync.dma_start(out=xs1[:], in_=xT[128:256, g0 * 128:(g0 + gt) * 128])
                gcomb = xp.tile([128, gt * EW], BF, tag="gcomb")
                gcf32 = gcomb[:].bitcast(F32)
                adg = xp.tile([128, gt], F32, tag="adg")
                nc.vector.memset(gcomb[:], 0.0)
                nc.vector.memset(
                    gcomb[:].rearrange("p (t k) -> p t k", k=EW)[:, :, H + 2:H + 3], 1.0)
                for i in range(gt):
                    hp = pp.tile([128, H], F32, tag="hp")
                    nc.tensor.matmul(out=hp[:], lhsT=xs0[:, i * 128:(i + 1) * 128],
                                     rhs=w1a[:], start=True, stop=False)
                    nc.tensor.matmul(out=hp[:], lhsT=xs1[:, i * 128:(i + 1) * 128],
                                     rhs=w1b[:], start=False, stop=True)
                    nc.scalar.activation(out=gcomb[:, i * EW:i * EW + H], in_=hp[:],
                                         func=AF.Copy)
                    scr = wp.tile([128, H], F32, tag="scr")
                    ascol = wp.tile([128, 1], F32, tag="ascol")
                    nc.vector.scalar_tensor_tensor(
                        out=scr[:], in0=hp[:], scalar=1.0, in1=a1s_r[:],
                        op0=ALU.mult, op1=ALU.mult, accum_out=ascol[:])
                    nc.vector.scalar_tensor_tensor(
                        out=scr[:], in0=hp[:], scalar=1.0, in1=a1d_r[:],
                        op0=ALU.mult, op1=ALU.mult, accum_out=adg[:, i:i + 1])
                    nc.vector.tensor_copy(out=gcf32[:, i * (EW // 2) + H // 2:
                                                    i * (EW // 2) + H // 2 + 1],
                                          in_=ascol[:])
                nc.sync.dma_start(
                    out=comb1_l[:].rearrange("(t p) k -> p t k", p=128)
                    [:, g0:g0 + gt, :], in_=gcomb[:])
                nc.sync.dma_start(
                    out=ad1_l[:].rearrange("(t p) one -> p t one", p=128)
                    [:, g0:g0 + gt, :], in_=adg[:])

            # ---- phase 2: all-gather layer-1 table ----
            nc.gpsimd.collective_compute(
                "AllGather", ALU.bypass, replica_groups=[list(range(N_CORES))],
                ins=[comb1_l[:].opt()], outs=[comb1_f[:].opt()])

            def edge_layer(comb_f, comb_l, ad_l, FW, last,
                           ans_r, and_r, bias_r, comb_out, ad_out):
                FWU = FW + 3        # h | as(2) | one
                qi = 0
                for g in groups:
                    tiles = g["tiles"]; gt = len(tiles); g0 = tiles[0]
                    ngc = g["n"]; c0 = g["start"]
                    ohg = ohp.tile([128, ngc * 128], BF, tag="ohg")
                    nc.sync.dma_start(out=ohg[:],
                                      in_=ohd[:, c0 * 128:(c0 + ngc) * 128])
                    gbuf = gp.tile([128, ngc * 128], BF, tag="gbuf")
                    for (s_r, n_r), rbase in zip(g["spans"],
                                                 range(0, NFULL, RANGE)):
                        # HW limit: ~1024 idx per dma_gather (128B/partition
                        # of wrapped idx data); split spans into <=8-chunk ops
                        for p0 in range(0, n_r, 8):
                            pn = min(8, n_r - p0)
                            s_p = s_r + p0
                            gview = gbuf[:, (s_p - c0) * 128:(s_p - c0 + pn) * 128] \
                                .rearrange("p (c k) -> p c k", k=128)
                            nc.gpsimd.dma_gather(
                                gview, comb_f[rbase:rbase + RANGE, :],
                                idxt[:, s_p * 8:(s_p + pn) * 8],
                                pn * 128, pn * 128, EW, queue_num=qi % 4)
                            qi += 1
                    # self-loop rows: sequential load of local table rows
                    gself = gp.tile([128, gt * EW], BF, tag="gself")
                    nc.sync.dma_start(
                        out=gself[:],
                        in_=comb_l[:].rearrange("(t p) k -> p t k", p=128)
                        [:, g0:g0 + gt, :])
                    # ad tiles: [128, gt*128] broadcast (free axis) + diag col
                    adw = ep.tile([128, gt * 128], F32, tag="adw")
                    nc.sync.dma_start(
                        out=adw[:],
                        in_=ad_l[g0 * 128:(g0 + gt) * 128, 0:1]
                        .rearrange("n one -> one n").to_broadcast([128, gt * 128]))
                    adc = ep.tile([128, gt], F32, tag="adc")
                    nc.sync.dma_start(
                        out=adc[:],
                        in_=ad_l[:].rearrange("(t p) one -> p t one", p=128)
                        [:, g0:g0 + gt, :])
                    # ad per edge (gathered chunks): fused onehot*ad + rowsum
                    ade = ep.tile([128, ngc], F32, tag="ade")
                    for t_i, t in enumerate(tiles):
                        for c in g["tile_cols"][t]:
                            osel = wp.tile([128, 128], F32, tag="osel")
                            nc.vector.scalar_tensor_tensor(
                                out=osel[:], in0=ohg[:, (c - c0) * 128:(c - c0 + 1) * 128],
                                scalar=1.0, in1=adw[:, t_i * 128:(t_i + 1) * 128],
                                op0=ALU.mult, op1=ALU.mult,
                                accum_out=ade[:, c - c0:c - c0 + 1])
                    # scores -> ex  (gathered chunks)
                    gf32 = gbuf[:].bitcast(F32).rearrange(
                        "p (c k) -> p c k", k=EW // 2)
                    asv = gf32[:, :, FW // 2:FW // 2 + 1].squeeze(2)
                    et = ep.tile([128, ngc], F32, tag="et")
                    nc.vector.tensor_tensor(out=et[:], in0=asv, in1=ade[:], op=ALU.add)
                    lrt = ep.tile([128, ngc], F32, tag="lrt")
                    nc.vector.scalar_tensor_tensor(
                        out=lrt[:], in0=et[:], scalar=NEG_SLOPE, in1=et[:],
                        op0=ALU.mult, op1=ALU.max)
                    ext = ep.tile([128, ngc], F32, tag="ext")
                    nc.scalar.activation(out=ext[:], in_=lrt[:], func=AF.Exp)
                    # scores -> ex  (self chunks)
                    gsf32 = gself[:].bitcast(F32).rearrange(
                        "p (t k) -> p t k", k=EW // 2)
                    asv_s = gsf32[:, :, FW // 2:FW // 2 + 1].squeeze(2)
                    ets = ep.tile([128, gt], F32, tag="ets")
                    nc.vector.tensor_tensor(out=ets[:], in0=asv_s, in1=adc[:], op=ALU.add)
                    lrs = ep.tile([128, gt], F32, tag="lrs")
                    nc.vector.scalar_tensor_tensor(
                        out=lrs[:], in0=ets[:], scalar=NEG_SLOPE, in1=ets[:],
                        op0=ALU.mult, op1=ALU.max)
                    exs = ep.tile([128, gt], F32, tag="exs")
                    nc.scalar.activation(out=exs[:], in_=lrs[:], func=AF.Exp)

                    # per-tile: weight + scatter-accumulate + epilogue
                    if last:
                        gout = ep.tile([128, gt * O], F32, tag="gout")
                    else:
                        gc2 = ep.tile([128, gt * EW], BF, tag="gc2")
                        gc2f = gc2[:].bitcast(F32)
                        adg2 = ep.tile([128, gt], F32, tag="adg2")
                        nc.vector.memset(gc2[:], 0.0)
                        nc.vector.memset(
                            gc2[:].rearrange("p (t k) -> p t k", k=EW)
                            [:, :, O + 2:O + 3], 1.0)
                    for t_i, t in enumerate(tiles):
                        ps = pp.tile([128, FWU], F32, tag="pe")
                        wts = wp.tile([128, FWU], BF, tag="wts")
                        nc.scalar.activation(
                            out=wts[:], in_=gself[:, t_i * EW:t_i * EW + FWU],
                            func=AF.Copy, scale=exs[:, t_i:t_i + 1])
                        nc.tensor.matmul(out=ps[:], lhsT=identb[:], rhs=wts[:],
                                         start=True, stop=False)
                        cols = g["tile_cols"][t]
                        for j, c in enumerate(cols):
                            wt = wp.tile([128, FWU], BF, tag="wt")
                            nc.scalar.activation(
                                out=wt[:], in_=gbuf[:, (c - c0) * 128:(c - c0) * 128 + FWU],
                                func=AF.Copy, scale=ext[:, c - c0:c - c0 + 1])
                            nc.tensor.matmul(
                                out=ps[:], lhsT=ohg[:, (c - c0) * 128:(c - c0 + 1) * 128],
                                rhs=wt[:], start=False, stop=(j == len(cols) - 1))
                        # epilogue
                        rec = wp.tile([128, 1], F32, tag="rec")
                        nc.vector.reciprocal(out=rec[:], in_=ps[:, FWU - 1:FWU])
                        if last:
                            nc.vector.scalar_tensor_tensor(
                                out=gout[:, t_i * O:(t_i + 1) * O], in0=ps[:, 0:FW],
                                scalar=rec[:], in1=bias_r[:],
                                op0=ALU.mult, op1=ALU.add)
                            continue
                        o1 = wp.tile([128, FW], F32, tag="o1")
                        nc.vector.scalar_tensor_tensor(
                            out=o1[:], in0=ps[:, 0:FW], scalar=rec[:],
                            in1=bias_r[:], op0=ALU.mult, op1=ALU.add)
                        o1b = wp.tile([128, FW], BF, tag="o1b")
                        nc.scalar.activation(out=o1b[:], in_=o1[:], func=AF.Relu)
                        trp = pp.tile([FW, 128], BF, tag="tr")
                        nc.tensor.transpose(out=trp[:], in_=o1b[:], identity=identb[:])
                        o1T = wp.tile([FW, 128], BF, tag="o1T")
                        nc.vector.tensor_copy(out=o1T[:], in_=trp[:])
                        h2p = pp.tile([128, O], F32, tag="h2")
                        nc.tensor.matmul(out=h2p[:], lhsT=o1T[:], rhs=w2t[:],
                                         start=True, stop=True)
                        nc.scalar.activation(out=gc2[:, t_i * EW:t_i * EW + O],
                                             in_=h2p[:], func=AF.Copy)
                        scr2 = wp.tile([128, O], F32, tag="scr2")
                        as2 = wp.tile([128, 1], F32, tag="as2")
                        nc.vector.scalar_tensor_tensor(
                            out=scr2[:], in0=h2p[:], scalar=1.0, in1=ans_r[:],
                            op0=ALU.mult, op1=ALU.mult, accum_out=as2[:])
                        nc.vector.scalar_tensor_tensor(
                            out=scr2[:], in0=h2p[:], scalar=1.0, in1=and_r[:],
                            op0=ALU.mult, op1=ALU.mult,
                            accum_out=adg2[:, t_i:t_i + 1])
                        nc.vector.tensor_copy(
                            out=gc2f[:, t_i * (EW // 2) + O // 2:
                                     t_i * (EW // 2) + O // 2 + 1], in_=as2[:])
                    if last:
                        nc.sync.dma_start(
                            out=outp[:].rearrange("(t p) k -> p t k", p=128)
                            [:, g0:g0 + gt, :], in_=gout[:])
                    else:
                        nc.sync.dma_start(
                            out=comb_out[:].rearrange("(t p) k -> p t k", p=128)
                            [:, g0:g0 + gt, :], in_=gc2[:])
                        nc.sync.dma_start(
                            out=ad_out[:].rearrange("(t p) one -> p t one", p=128)
                            [:, g0:g0 + gt, :], in_=adg2[:])

            # ---- phase 3: edge layer 1 (fused layer-2 GEMM) ----
            edge_layer(comb1_f, comb1_l, ad1_l, H, False,
                       a2s_r, a2d_r, b1_r, comb2_l, ad2_l)

            # ---- phase 4: all-gather layer-2 table ----
            nc.gpsimd.collective_compute(
                "AllGather", ALU.bypass, replica_groups=[list(range(N_CORES))],
                ins=[comb2_l[:].opt()], outs=[comb2_f[:].opt()])

            # ---- phase 5: edge layer 2 ----
            edge_layer(comb2_f, comb2_l, ad2_l, O, True,
                       None, None, b2_r, None, None)

    nc.compile()
    return nc


def kernel(x, edge_index, W1, a1_src, a1_dst, b1, W2, a2_src, a2_dst, b2):
    x = np.asarray(x, dtype=np.float32)
    staged, groups, totch = _prep_edges(np.asarray(edge_index))
    nc = _build(groups, totch)

    ident = np.eye(128, dtype=ml_dtypes.bfloat16)
    common = dict(
        w1=np.asarray(W1, np.float32).astype(ml_dtypes.bfloat16),
        w2=np.asarray(W2, np.float32).astype(ml_dtypes.bfloat16),
        a1s=np.asarray(a1_src, np.float32), a1d=np.asarray(a1_dst, np.float32),
        b1=np.asarray(b1, np.float32), a2s=np.asarray(a2_src, np.float32),
        a2d=np.asarray(a2_dst, np.float32), b2=np.asarray(b2, np.float32),
        identd=ident,
    )
    in_maps = []
    for c in range(N_CORES):
        idx16, oh = staged[c]
        xT = np.zeros((F1, NPAD), ml_dtypes.bfloat16)
        xT[:, :NPC] = x[c * NPC:(c + 1) * NPC].T.astype(ml_dtypes.bfloat16)
        in_maps.append(dict(common, xT=xT, idxs=idx16, ohd=oh))

    global _LAST_NC, _LAST_INMAPS
    _LAST_NC, _LAST_INMAPS = nc, in_maps
    res = run_bass_kernel_spmd(nc, in_maps, core_ids=list(range(N_CORES)))
    out = np.concatenate(
        [res.results[c]["outp"][:NPC] for c in range(N_CORES)], axis=0)
    return out.astype(np.float32)
